# revision 1
# baseline (speedup 1.0000x reference)
"""Trainium2 Bass kernel for Conv2d_XnorPP_SCA (binarized 3x3 conv).

Computes: out = conv2d(sign(x), round(tanh(w)), stride=1, pad=1) * alpha
  x: [64, 64, 112, 112] f32, w: [64, 64, 3, 3] f32, alpha: [64,1,1] f32

Strategy (per NeuronCore, data-parallel over batch, 8 images/core):
  - Zero-padded flat layout: each image is sign-binarized (bf16) into a
    [64, 114*114] SBUF tile with zero borders; every 3x3 tap then becomes a
    constant column offset, so the conv is 9 PSUM-accumulated matmuls
    (K=Cin=64, M=Cout=64) per 4-row output block.
  - Two images resident at once (partitions 0-63 / 64-127). Matmuls are
    issued with explicit tile_position so the 4 (image x row-half) streams
    occupy the 4 PE 64x64 quadrants CONCURRENTLY (measured ~10x vs serial).
  - Output rows are split top-half/bottom-half (rh) so each image's result
    stages as [128=(rh,c), 56*112] fp16 and leaves in ONE 1.6MB DMA with
    12.5KB contiguous per partition. fp16 is exact: outputs are integers
    bounded by 576 < 2048.
  - alpha is folded into the (ternary, exactly bf16-representable) weights.
"""

import numpy as np
import ml_dtypes

H = W = 112
WP = 114
P_COLS = WP * WP + 2  # 12998: +1 margin at each end
CIN = COUT = 64
N_CORES = 8
NI = 8  # images per core
ROWS_PER_CHUNK = 28  # input load/sign granularity
NJ = 14  # 4-row blocks per output half (56 rows per half)


def build_nc(ni=NI, rep=1, skip_mm=False, skip_out=False, skip_in=False):
    import concourse.bacc as bacc
    import concourse.mybir as mybir
    from concourse.tile import TileContext

    f32 = mybir.dt.float32
    bf16 = mybir.dt.bfloat16
    fp16 = mybir.dt.float16

    nc = bacc.Bacc("TRN2", target_bir_lowering=False, debug=False)
    x_d = nc.dram_tensor("x", [ni, CIN, H, W], f32, kind="ExternalInput")
    w_d = nc.dram_tensor("w", [128, 9 * COUT], bf16, kind="ExternalInput")
    o_d = nc.dram_tensor("out", [ni, COUT, H, W], fp16, kind="ExternalOutput")

    x_flat = x_d.ap().rearrange("n c h w -> (n c) (h w)")
    npairs = ni // 2
    n_chunks = H // ROWS_PER_CHUNK  # 4

    with TileContext(nc) as tc:
        with (
            tc.tile_pool(name="wp", bufs=1) as wp,
            tc.tile_pool(name="inp", bufs=3) as inp,
            tc.tile_pool(name="pp", bufs=1) as pp,
            tc.tile_pool(name="op", bufs=2) as op,
            tc.tile_pool(name="psp", bufs=8, space="PSUM") as psp,
        ):
            w_sb = wp.tile([128, 9 * COUT], bf16, name="w_sb")
            nc.sync.dma_start(out=w_sb[:, :], in_=w_d.ap())

            p_tiles = []
            for i in range(2):
                pt = pp.tile([128, P_COLS], bf16, tag=f"p{i}", name=f"p{i}")
                nc.vector.memset(pt[:, :], 0.0)
                p_tiles.append(pt)

            for r in range(rep):
                for pair in range(npairs):
                    p = p_tiles[pair % 2]
                    # ---- load x for both images, binarize into p ----
                    for ci in range(n_chunks if not skip_in else 0):
                        y0 = ci * ROWS_PER_CHUNK
                        st = inp.tile([128, ROWS_PER_CHUNK * W], f32,
                                      tag="xin", name="xin")
                        nc.sync.dma_start(
                            out=st[:, :],
                            in_=x_flat[pair * 128:(pair + 1) * 128,
                                       y0 * W:(y0 + ROWS_PER_CHUNK) * W],
                        )
                        dst = p[:, 116 + y0 * WP:
                                116 + y0 * WP + ROWS_PER_CHUNK * WP]
                        dst = dst.rearrange("q (r w) -> q r w", w=WP)[:, :, :W]
                        src = st[:, :].rearrange("q (r w) -> q r w", w=W)
                        nc.scalar.activation(
                            out=dst, in_=src,
                            func=mybir.ActivationFunctionType.Sign)

                    # ---- output staging: [128=(rh,c), 56*112] fp16/img ----
                    st_out = []
                    for ii in range(2):
                        so = op.tile([128, NJ * 4 * W], fp16,
                                     tag=f"so{ii}", name=f"so{ii}")
                        st_out.append(so)

                    # ---- conv: 14 j-blocks x (2 img x 2 rh) quadrants ----
                    for j in range(NJ):
                        q_tiles = []
                        if not skip_mm:
                            for img in range(2):
                                qt = psp.tile([128, 456], f32, tag="ps",
                                              name=f"ps{img}",
                                              padded_shape=[128, 512])
                                q_tiles.append(qt)
                        for t in range(9 if not skip_mm else 0):
                            ky, kx = divmod(t, 3)
                            first, last = (t == 0), (t == 8)
                            for img in range(2):
                                lhs = w_sb[64 * img:64 * (img + 1),
                                           t * 64:(t + 1) * 64]
                                for rh in range(2):
                                    y0 = 4 * j + 56 * rh
                                    s = (y0 + ky) * WP + kx
                                    nc.tensor.matmul(
                                        q_tiles[img][64 * rh:64 * (rh + 1),
                                                     0:456],
                                        lhs,
                                        p[64 * img:64 * (img + 1), s:s + 456],
                                        start=first, stop=last,
                                        skip_group_check=True,
                                        tile_position=(64 * img, 64 * rh))
                        # evacuate both halves to fp16 staging (DVE)
                        if not skip_out:
                            for img in range(2):
                                if skip_mm:
                                    src = p[:, j * 456:(j + 1) * 456]
                                    src = src.rearrange("q (r w) -> q r w",
                                                        w=WP)[:, :, 1:1 + W]
                                else:
                                    src = q_tiles[img][:, 0:456]
                                    src = src.rearrange("q (r w) -> q r w",
                                                        w=WP)
                                    src = src[:, :, 1:1 + W]
                                dst = st_out[img][:, j * 4 * W:(j + 1) * 4 * W]
                                dst = dst.rearrange("q (r w) -> q r w", w=W)
                                nc.vector.tensor_copy(out=dst, in_=src)

                    # ---- DMA out: one per (image, row-half) ----
                    if not skip_out:
                        for img in range(2):
                            n = pair * 2 + img
                            for rh in range(2):
                                dst = o_d.ap()[n][:, 56 * rh:56 * (rh + 1), :]
                                dst = dst.rearrange("c r w -> c (r w)")
                                nc.sync.dma_start(
                                    out=dst,
                                    in_=st_out[img][64 * rh:64 * (rh + 1), :])
    nc.compile()
    return nc


def pack_weights(weight, alpha):
    """Ternarize (round(tanh(w))), fold alpha, pack as [128, 9*64] bf16 lhsT."""
    wt = _ternarize(np.asarray(weight, dtype=np.float32))
    wt = wt * np.asarray(alpha, dtype=np.float32).reshape(-1, 1, 1, 1)
    # lhsT[k=cin, t*64+cout]
    arr = wt.transpose(1, 2, 3, 0).reshape(CIN, 9 * COUT)
    pack = np.empty((128, 9 * COUT), dtype=ml_dtypes.bfloat16)
    pack[0:64] = arr.astype(ml_dtypes.bfloat16)
    pack[64:128] = pack[0:64]
    return pack


def _ternarize(w):
    try:
        import jax
        cpu = jax.devices("cpu")[0]
        with jax.default_device(cpu):
            import jax.numpy as jnp
            return np.asarray(jnp.round(jnp.tanh(jnp.asarray(w))))
    except Exception:
        return np.round(np.tanh(w.astype(np.float32))).astype(np.float32)


_NC_CACHE = {}


def _get_nc(rep=1):
    key = f"nc{rep}"
    if key not in _NC_CACHE:
        _NC_CACHE[key] = build_nc(NI, rep=rep)
    return _NC_CACHE[key]


def _make_runner(rep=1, donate=True):
    """Build (once) a jitted shard_map callable running the NEFF on 8 cores."""
    key = f"runner{rep}_{donate}"
    if key in _NC_CACHE:
        return _NC_CACHE[key]
    import jax
    import concourse.mybir as mybir
    from concourse import bass2jax
    from jax.sharding import Mesh, PartitionSpec
    from jax.experimental.shard_map import shard_map

    nc = _get_nc(rep)
    bass2jax.install_neuronx_cc_hook()

    partition_name = (nc.partition_id_tensor.name
                      if nc.partition_id_tensor else None)
    in_names, out_names, out_avals, zero_shapes = [], [], [], []
    for alloc in nc.m.functions[0].allocations:
        if not isinstance(alloc, mybir.MemoryLocationSet):
            continue
        name = alloc.memorylocations[0].name
        if alloc.kind == "ExternalInput":
            if name != partition_name:
                in_names.append(name)
        elif alloc.kind == "ExternalOutput":
            out_names.append(name)
            shape = tuple(alloc.tensor_shape)
            dtype = mybir.dt.np(alloc.dtype)
            out_avals.append(jax.core.ShapedArray(shape, dtype))
            zero_shapes.append((shape, dtype))
    n_params = len(in_names)
    all_in_names = in_names + out_names
    if partition_name is not None:
        all_in_names = all_in_names + [partition_name]

    def _body(*args):
        operands = list(args)
        if partition_name is not None:
            operands.append(bass2jax.partition_id_tensor())
        outs = bass2jax._bass_exec_p.bind(
            *operands,
            out_avals=tuple(out_avals),
            in_names=tuple(all_in_names),
            out_names=tuple(out_names),
            lowering_input_output_aliases=(),
            sim_require_finite=True,
            sim_require_nnan=True,
            nc=nc,
        )
        return tuple(outs)

    devices = jax.devices()[:N_CORES]
    mesh = Mesh(np.asarray(devices), ("core",))
    n_outs = len(out_names)
    donate_idx = tuple(range(n_params, n_params + n_outs)) if donate else ()
    in_specs = (PartitionSpec("core"),) * (n_params + n_outs)
    out_specs = (PartitionSpec("core"),) * n_outs
    sharded = jax.jit(
        shard_map(_body, mesh=mesh, in_specs=in_specs, out_specs=out_specs,
                  check_rep=False),
        donate_argnums=donate_idx, keep_unused=True)
    runner = {
        "fn": sharded, "mesh": mesh, "in_names": in_names,
        "out_names": out_names, "zero_shapes": zero_shapes,
        "n_params": n_params,
    }
    _NC_CACHE[key] = runner
    return runner


def make_concat_inputs(x, w_pack):
    """Per-core inputs concatenated on axis 0 (shard_map layout)."""
    xs = np.ascontiguousarray(x.reshape(N_CORES * NI, CIN, H, W))
    ws = np.concatenate([w_pack] * N_CORES, axis=0)
    return {"x": xs, "w": ws}


def make_zeros(rep=1):
    r = _make_runner(rep)
    return [np.zeros((N_CORES * s[0], *s[1:]), d) for s, d in r["zero_shapes"]]


def run_concat(concat_by_name, zeros=None, rep=1):
    """Run on 8 cores. Inputs may be numpy or device-resident jax arrays."""
    r = _make_runner(rep)
    if zeros is None:
        zeros = make_zeros(rep)
    args = [concat_by_name[n] for n in r["in_names"]] + list(zeros)
    out_arrs = r["fn"](*args)
    return out_arrs


def kernel(x, weight, alpha):
    x = np.asarray(x, dtype=np.float32)
    w_pack = pack_weights(weight, alpha)
    concat = make_concat_inputs(x, w_pack)
    out_arrs = run_concat(concat)
    out = np.asarray(out_arrs[0]).reshape(64, COUT, H, W)
    return out.astype(np.float32)



# revision 12
# speedup vs baseline: 1.1898x; 1.1898x over previous
"""Trainium2 Bass kernel for Conv2d_XnorPP_SCA (binarized 3x3 conv).

Computes: out = conv2d(sign(x), round(tanh(w)), stride=1, pad=1) * alpha
  x: [64, 64, 112, 112] f32, w: [64, 64, 3, 3] f32, alpha: [64,1,1] f32

Strategy (per NeuronCore, data-parallel over batch, 8 images/core):
  - Zero-padded flat layout: each image is sign-binarized (bf16) into a
    [64, 114*114] SBUF tile with zero borders; every 3x3 tap then becomes a
    constant column offset, so the conv is 9 PSUM-accumulated matmuls
    (K=Cin=64, M=Cout=64) per 4-row output block.
  - Two images resident at once (partitions 0-63 / 64-127). Matmuls are
    issued with explicit tile_position so the 4 (image x row-half) streams
    occupy the 4 PE 64x64 quadrants CONCURRENTLY (measured ~10x vs serial).
  - Output rows are split top-half/bottom-half (rh) so each image's result
    stages as [128=(rh,c), 56*112] fp16 and leaves in ONE 1.6MB DMA with
    12.5KB contiguous per partition. fp16 is exact: outputs are integers
    bounded by 576 < 2048.
  - alpha is folded into the (ternary, exactly bf16-representable) weights.
"""

import numpy as np
import ml_dtypes

H = W = 112
WP = 114
P_COLS = WP * WP + 2  # 12998: +1 margin at each end
CIN = COUT = 64
N_CORES = 8
NI = 8  # images per core
ROWS_PER_CHUNK = 28  # input load/sign granularity
NJ = 14  # 4-row blocks per output half (56 rows per half)
USE_V2 = False  # fp8 DoubleRow variant (DR blocked on col groups 64+)


def build_nc(ni=NI, rep=1, skip_mm=False, skip_out=False, skip_in=False,
             bf16_x=True):
    import concourse.bacc as bacc
    import concourse.mybir as mybir
    from concourse.tile import TileContext

    f32 = mybir.dt.float32
    bf16 = mybir.dt.bfloat16
    fp16 = mybir.dt.float16

    x_dt = bf16 if bf16_x else f32
    nc = bacc.Bacc("TRN2", target_bir_lowering=False, debug=False)
    x_d = nc.dram_tensor("x", [ni, CIN, H, W], x_dt, kind="ExternalInput")
    w_d = nc.dram_tensor("w", [128, 9 * COUT], bf16, kind="ExternalInput")
    o_d = nc.dram_tensor("out", [ni, COUT, H, W], fp16, kind="ExternalOutput")

    x_flat = x_d.ap().rearrange("n c h w -> (n c) (h w)")
    npairs = ni // 2
    n_chunks = H // ROWS_PER_CHUNK  # 4

    with TileContext(nc) as tc:
        with (
            tc.tile_pool(name="wp", bufs=1) as wp,
            tc.tile_pool(name="inp", bufs=3) as inp,
            tc.tile_pool(name="pp", bufs=1) as pp,
            tc.tile_pool(name="op", bufs=2) as op,
            tc.tile_pool(name="psp", bufs=8, space="PSUM") as psp,
        ):
            w_sb = wp.tile([128, 9 * COUT], bf16, name="w_sb")
            nc.sync.dma_start(out=w_sb[:, :], in_=w_d.ap())

            p_tiles = []
            for i in range(2):
                pt = pp.tile([128, P_COLS], bf16, tag=f"p{i}", name=f"p{i}")
                nc.vector.memset(pt[:, :], 0.0)
                p_tiles.append(pt)

            for r in range(rep):
                for pair in range(npairs):
                    p = p_tiles[pair % 2]
                    # ---- load x for both images, binarize into p ----
                    for ci in range(n_chunks if not skip_in else 0):
                        y0 = ci * ROWS_PER_CHUNK
                        st = inp.tile([128, ROWS_PER_CHUNK * W], x_dt,
                                      tag="xin", name="xin")
                        nc.sync.dma_start(
                            out=st[:, :],
                            in_=x_flat[pair * 128:(pair + 1) * 128,
                                       y0 * W:(y0 + ROWS_PER_CHUNK) * W],
                        )
                        dst = p[:, 116 + y0 * WP:
                                116 + y0 * WP + ROWS_PER_CHUNK * WP]
                        dst = dst.rearrange("q (r w) -> q r w", w=WP)[:, :, :W]
                        src = st[:, :].rearrange("q (r w) -> q r w", w=W)
                        nc.scalar.activation(
                            out=dst, in_=src,
                            func=mybir.ActivationFunctionType.Sign)

                    # ---- output staging: [128=(rh,c), 56*112] fp16/img ----
                    st_out = []
                    for ii in range(2):
                        so = op.tile([128, NJ * 4 * W], fp16,
                                     tag=f"so{ii}", name=f"so{ii}")
                        st_out.append(so)

                    # ---- conv: 14 j-blocks x (2 img x 2 rh) quadrants ----
                    for j in range(NJ):
                        q_tiles = []
                        if not skip_mm:
                            for img in range(2):
                                qt = psp.tile([128, 456], f32, tag="ps",
                                              name=f"ps{img}",
                                              padded_shape=[128, 512])
                                q_tiles.append(qt)
                        for t in range(9 if not skip_mm else 0):
                            ky, kx = divmod(t, 3)
                            first, last = (t == 0), (t == 8)
                            for img in range(2):
                                lhs = w_sb[64 * img:64 * (img + 1),
                                           t * 64:(t + 1) * 64]
                                for rh in range(2):
                                    y0 = 4 * j + 56 * rh
                                    s = (y0 + ky) * WP + kx
                                    nc.tensor.matmul(
                                        q_tiles[img][64 * rh:64 * (rh + 1),
                                                     0:456],
                                        lhs,
                                        p[64 * img:64 * (img + 1), s:s + 456],
                                        start=first, stop=last,
                                        skip_group_check=True,
                                        tile_position=(64 * img, 64 * rh))
                        # evacuate both halves to fp16 staging (DVE)
                        if not skip_out:
                            for img in range(2):
                                if skip_mm:
                                    src = p[:, j * 456:(j + 1) * 456]
                                    src = src.rearrange("q (r w) -> q r w",
                                                        w=WP)[:, :, 1:1 + W]
                                else:
                                    src = q_tiles[img][:, 0:456]
                                    src = src.rearrange("q (r w) -> q r w",
                                                        w=WP)
                                    src = src[:, :, 1:1 + W]
                                dst = st_out[img][:, j * 4 * W:(j + 1) * 4 * W]
                                dst = dst.rearrange("q (r w) -> q r w", w=W)
                                nc.vector.tensor_copy(out=dst, in_=src)

                    # ---- DMA out: one per (image, row-half) ----
                    if not skip_out:
                        for img in range(2):
                            n = pair * 2 + img
                            for rh in range(2):
                                dst = o_d.ap()[n][:, 56 * rh:56 * (rh + 1), :]
                                dst = dst.rearrange("c r w -> c (r w)")
                                nc.sync.dma_start(
                                    out=dst,
                                    in_=st_out[img][64 * rh:64 * (rh + 1), :])
    nc.compile()
    return nc


def build_nc_v2(ni=NI, rep=1, skip_mm=False, skip_out=False, skip_in=False,
                act_every=4, dr=True):
    """fp8 DoubleRow variant: 4 paired-tap DR matmuls + 1 single per block.

    - input DMA casts f32->bf16 (SWDGE), ACT sign bf16->fp8e4 into p
    - weights fp8e4 [128, 9*64] (no alpha fold); alpha applied in evacuation
    - evacuation split DVE (tensor_scalar mul) / ACT (activation Copy+scale),
      ACT takes every `act_every`-th copy
    """
    import concourse.bacc as bacc
    import concourse.mybir as mybir
    from concourse.tile import TileContext
    from concourse.ap import AP

    f32 = mybir.dt.float32
    bf16 = mybir.dt.bfloat16
    fp16 = mybir.dt.float16
    fp8 = mybir.dt.float8e4

    nc = bacc.Bacc("TRN2", target_bir_lowering=False, debug=False)
    x_d = nc.dram_tensor("x", [ni, CIN, H, W], f32, kind="ExternalInput")
    w_d = nc.dram_tensor("w", [128, 9 * COUT], fp8, kind="ExternalInput")
    a_d = nc.dram_tensor("a", [128, 1], f32, kind="ExternalInput")
    o_d = nc.dram_tensor("out", [ni, COUT, H, W], fp16, kind="ExternalOutput")

    x_flat = x_d.ap().rearrange("n c h w -> (n c) (h w)")
    npairs = ni // 2
    n_chunks = H // ROWS_PER_CHUNK  # 4

    # tap order: t = 3*ky + kx ; offset within flat image = ky*WP + kx
    # DR pairs (consecutive taps): (0,1) d=1, (2,3) d=WP-2, (4,5) d=1,
    # (6,7) d=1 ; single tap 8.
    PAIRS = [(0, 1), (2, WP - 2), (4, 1), (6, 1)]
    TAP_OFF = [(t // 3) * WP + (t % 3) for t in range(9)]

    with TileContext(nc) as tc:
        with (
            tc.tile_pool(name="wp", bufs=1) as wp,
            tc.tile_pool(name="inp", bufs=3) as inp,
            tc.tile_pool(name="pp", bufs=1) as pp,
            tc.tile_pool(name="op", bufs=2) as op,
            tc.tile_pool(name="psp", bufs=8, space="PSUM") as psp,
        ):
            w_sb = wp.tile([128, 9 * COUT], fp8, name="w_sb")
            nc.sync.dma_start(out=w_sb[:, :], in_=w_d.ap())
            a_sb = wp.tile([128, 1], f32, name="a_sb")
            nc.sync.dma_start(out=a_sb[:, :], in_=a_d.ap())

            p_tiles = []
            for i in range(2):
                pt = pp.tile([128, P_COLS], fp8, tag=f"p{i}", name=f"p{i}")
                nc.vector.memset(pt[:, :], 0.0)
                p_tiles.append(pt)

            copy_idx = 0
            for r in range(rep):
                for pair in range(npairs):
                    p = p_tiles[pair % 2]
                    # ---- load x (cast to bf16), binarize to fp8 into p ----
                    for ci in range(n_chunks if not skip_in else 0):
                        y0 = ci * ROWS_PER_CHUNK
                        st = inp.tile([128, ROWS_PER_CHUNK * W], bf16,
                                      tag="xin", name="xin")
                        nc.gpsimd.dma_start(
                            out=st[:, :],
                            in_=x_flat[pair * 128:(pair + 1) * 128,
                                       y0 * W:(y0 + ROWS_PER_CHUNK) * W],
                        )
                        dst = p[:, 116 + y0 * WP:
                                116 + y0 * WP + ROWS_PER_CHUNK * WP]
                        dst = dst.rearrange("q (r w) -> q r w", w=WP)[:, :, :W]
                        src = st[:, :].rearrange("q (r w) -> q r w", w=W)
                        nc.scalar.activation(
                            out=dst, in_=src,
                            func=mybir.ActivationFunctionType.Sign)

                    # ---- output staging: [128=(rh,c), 56*112] fp16/img ----
                    st_out = []
                    for ii in range(2):
                        so = op.tile([128, NJ * 4 * W], fp16,
                                     tag=f"so{ii}", name=f"so{ii}")
                        st_out.append(so)

                    # ---- conv: 14 j-blocks x (2 img x 2 rh) quadrants ----
                    for j in range(NJ):
                        q_tiles = []
                        if not skip_mm:
                            for img in range(2):
                                qt = psp.tile([128, 456], f32, tag="ps",
                                              name=f"ps{img}",
                                              padded_shape=[128, 512])
                                q_tiles.append(qt)
                            for img in range(2):
                                for rh in range(2):
                                    y0 = 4 * j + 56 * rh
                                    s0 = y0 * WP
                                    pbase = img * 64 * P_COLS
                                    wbase = img * 64 * (9 * COUT)
                                    out_ap = q_tiles[img][
                                        64 * rh:64 * (rh + 1), 0:456]
                                    if dr:
                                        for pi, (t, d) in enumerate(PAIRS):
                                            rhs = AP(
                                                p.tensor,
                                                p[0:1, 0:1].offset + pbase
                                                + s0 + TAP_OFF[t],
                                                [[P_COLS, 64], [d, 2],
                                                 [1, 456]])
                                            lhsT = AP(
                                                w_sb.tensor,
                                                w_sb[0:1, 0:1].offset + wbase
                                                + t * COUT,
                                                [[9 * COUT, 64], [COUT, 2],
                                                 [1, COUT]])
                                            nc.tensor.matmul(
                                                out_ap, lhsT, rhs,
                                                start=(pi == 0), stop=False,
                                                perf_mode=(mybir.MatmulPerfMode
                                                           .DoubleRow),
                                                skip_group_check=True,
                                                tile_position=(64 * img,
                                                               64 * rh))
                                        lhsT8 = w_sb[64 * img:64 * (img + 1),
                                                     8 * COUT:9 * COUT]
                                        s8 = s0 + TAP_OFF[8]
                                        rhs8 = AP(
                                            p.tensor,
                                            p[0:1, 0:1].offset + pbase + s8,
                                            [[P_COLS, 64], [1, 456]])
                                        nc.tensor.matmul(
                                            out_ap, lhsT8, rhs8,
                                            start=False, stop=True,
                                            skip_group_check=True,
                                            tile_position=(64 * img, 64 * rh))
                                    else:
                                        for t in range(9):
                                            lhsTt = w_sb[
                                                64 * img:64 * (img + 1),
                                                t * COUT:(t + 1) * COUT]
                                            st_ = s0 + TAP_OFF[t]
                                            rhst = AP(
                                                p.tensor,
                                                p[0:1, 0:1].offset + pbase
                                                + st_,
                                                [[P_COLS, 64], [1, 456]])
                                            nc.tensor.matmul(
                                                out_ap, lhsTt, rhst,
                                                start=(t == 0), stop=(t == 8),
                                                skip_group_check=True,
                                                tile_position=(64 * img,
                                                               64 * rh))
                        # evacuate: alpha * psum -> fp16 staging (DVE/ACT mix)
                        if not skip_out:
                            for img in range(2):
                                if skip_mm:
                                    src = p[:, j * 456:(j + 1) * 456]
                                    src = src.rearrange("q (r w) -> q r w",
                                                        w=WP)[:, :, 1:1 + W]
                                else:
                                    src = q_tiles[img][:, 0:456]
                                    src = src.rearrange("q (r w) -> q r w",
                                                        w=WP)
                                    src = src[:, :, 1:1 + W]
                                dst = st_out[img][:, j * 4 * W:(j + 1) * 4 * W]
                                dst = dst.rearrange("q (r w) -> q r w", w=W)
                                if copy_idx % act_every == act_every - 1:
                                    nc.scalar.activation(
                                        out=dst, in_=src,
                                        func=(mybir.ActivationFunctionType
                                              .Copy),
                                        scale=a_sb[:, 0:1])
                                else:
                                    nc.vector.tensor_scalar_mul(
                                        dst, src, a_sb[:, 0:1])
                                copy_idx += 1

                    # ---- DMA out: one per (image, row-half) ----
                    if not skip_out:
                        for img in range(2):
                            n = pair * 2 + img
                            for rh in range(2):
                                dst = o_d.ap()[n][:, 56 * rh:56 * (rh + 1), :]
                                dst = dst.rearrange("c r w -> c (r w)")
                                nc.sync.dma_start(
                                    out=dst,
                                    in_=st_out[img][64 * rh:64 * (rh + 1), :])
    nc.compile()
    return nc


def pack_weights_v2(weight):
    """Ternarize (round(tanh(w))), pack as [128, 9*64] fp8e4 lhsT (no alpha)."""
    wt = _ternarize(np.asarray(weight, dtype=np.float32))
    arr = wt.transpose(1, 2, 3, 0).reshape(CIN, 9 * COUT)
    pack = np.empty((128, 9 * COUT), dtype=ml_dtypes.float8_e4m3)
    pack[0:64] = arr.astype(ml_dtypes.float8_e4m3)
    pack[64:128] = pack[0:64]
    return pack


def pack_alpha_v2(alpha):
    a = np.asarray(alpha, dtype=np.float32).reshape(-1)
    out = np.empty((128, 1), np.float32)
    out[0:64, 0] = a
    out[64:128, 0] = a
    return out


def pack_weights(weight, alpha):
    """Ternarize (round(tanh(w))), fold alpha, pack as [128, 9*64] bf16 lhsT."""
    wt = _ternarize(np.asarray(weight, dtype=np.float32))
    wt = wt * np.asarray(alpha, dtype=np.float32).reshape(-1, 1, 1, 1)
    # lhsT[k=cin, t*64+cout]
    arr = wt.transpose(1, 2, 3, 0).reshape(CIN, 9 * COUT)
    pack = np.empty((128, 9 * COUT), dtype=ml_dtypes.bfloat16)
    pack[0:64] = arr.astype(ml_dtypes.bfloat16)
    pack[64:128] = pack[0:64]
    return pack


def _ternarize(w):
    try:
        import jax
        cpu = jax.devices("cpu")[0]
        with jax.default_device(cpu):
            import jax.numpy as jnp
            return np.asarray(jnp.round(jnp.tanh(jnp.asarray(w))))
    except Exception:
        return np.round(np.tanh(w.astype(np.float32))).astype(np.float32)


_NC_CACHE = {}


def _get_nc(rep=1, **fl):
    key = ("nc", rep, tuple(sorted(fl.items())))
    if key not in _NC_CACHE:
        fl = dict(fl)
        builder = build_nc_v2 if fl.pop("v2", USE_V2) else build_nc
        _NC_CACHE[key] = builder(NI, rep=rep, **fl)
    return _NC_CACHE[key]


def _make_runner(rep=1, donate=True, **fl):
    """Build (once) a jitted shard_map callable running the NEFF on 8 cores."""
    key = ("runner", rep, donate, tuple(sorted(fl.items())))
    if key in _NC_CACHE:
        return _NC_CACHE[key]
    import jax
    import concourse.mybir as mybir
    from concourse import bass2jax
    from jax.sharding import Mesh, PartitionSpec
    from jax.experimental.shard_map import shard_map

    nc = _get_nc(rep, **fl)
    bass2jax.install_neuronx_cc_hook()

    partition_name = (nc.partition_id_tensor.name
                      if nc.partition_id_tensor else None)
    in_names, out_names, out_avals, zero_shapes = [], [], [], []
    for alloc in nc.m.functions[0].allocations:
        if not isinstance(alloc, mybir.MemoryLocationSet):
            continue
        name = alloc.memorylocations[0].name
        if alloc.kind == "ExternalInput":
            if name != partition_name:
                in_names.append(name)
        elif alloc.kind == "ExternalOutput":
            out_names.append(name)
            shape = tuple(alloc.tensor_shape)
            dtype = mybir.dt.np(alloc.dtype)
            out_avals.append(jax.core.ShapedArray(shape, dtype))
            zero_shapes.append((shape, dtype))
    n_params = len(in_names)
    all_in_names = in_names + out_names
    if partition_name is not None:
        all_in_names = all_in_names + [partition_name]

    def _body(*args):
        operands = list(args)
        if partition_name is not None:
            operands.append(bass2jax.partition_id_tensor())
        outs = bass2jax._bass_exec_p.bind(
            *operands,
            out_avals=tuple(out_avals),
            in_names=tuple(all_in_names),
            out_names=tuple(out_names),
            lowering_input_output_aliases=(),
            sim_require_finite=True,
            sim_require_nnan=True,
            nc=nc,
        )
        return tuple(outs)

    devices = jax.devices()[:N_CORES]
    mesh = Mesh(np.asarray(devices), ("core",))
    n_outs = len(out_names)
    donate_idx = tuple(range(n_params, n_params + n_outs)) if donate else ()
    in_specs = (PartitionSpec("core"),) * (n_params + n_outs)
    out_specs = (PartitionSpec("core"),) * n_outs
    sharded = jax.jit(
        shard_map(_body, mesh=mesh, in_specs=in_specs, out_specs=out_specs,
                  check_rep=False),
        donate_argnums=donate_idx, keep_unused=True)
    runner = {
        "fn": sharded, "mesh": mesh, "in_names": in_names,
        "out_names": out_names, "zero_shapes": zero_shapes,
        "n_params": n_params,
    }
    _NC_CACHE[key] = runner
    return runner


def make_concat_inputs(x, w_pack, a_pack=None):
    """Per-core inputs concatenated on axis 0 (shard_map layout)."""
    xs = np.ascontiguousarray(x.reshape(N_CORES * NI, CIN, H, W))
    ws = np.concatenate([w_pack] * N_CORES, axis=0)
    d = {"x": xs, "w": ws}
    if a_pack is not None:
        d["a"] = np.concatenate([a_pack] * N_CORES, axis=0)
    return d


def make_zeros(rep=1):
    r = _make_runner(rep)
    return [np.zeros((N_CORES * s[0], *s[1:]), d) for s, d in r["zero_shapes"]]


def run_concat(concat_by_name, zeros=None, rep=1):
    """Run on 8 cores. Inputs may be numpy or device-resident jax arrays."""
    r = _make_runner(rep)
    if zeros is None:
        zeros = make_zeros(rep)
    args = [concat_by_name[n] for n in r["in_names"]] + list(zeros)
    out_arrs = r["fn"](*args)
    return out_arrs


def make_concat_all(x, weight, alpha):
    """Concat inputs for the ACTIVE kernel variant (v1 or v2)."""
    x = np.asarray(x, dtype=np.float32)
    if USE_V2:
        return make_concat_inputs(x, pack_weights_v2(weight),
                                  pack_alpha_v2(alpha))
    # bf16 recode of x is sign-exact (bf16 keeps the f32 exponent range),
    # so the device computes the identical sign(x) conv.
    x = x.astype(ml_dtypes.bfloat16)
    return make_concat_inputs(x, pack_weights(weight, alpha))


def kernel(x, weight, alpha):
    concat = make_concat_all(x, weight, alpha)
    out_arrs = run_concat(concat)
    out = np.asarray(out_arrs[0]).reshape(64, COUT, H, W)
    return out.astype(np.float32)



# revision 13
# speedup vs baseline: 1.2222x; 1.0273x over previous
"""Trainium2 Bass kernel for Conv2d_XnorPP_SCA (binarized 3x3 conv).

Computes: out = conv2d(sign(x), round(tanh(w)), stride=1, pad=1) * alpha
  x: [64, 64, 112, 112] f32, w: [64, 64, 3, 3] f32, alpha: [64,1,1] f32

Strategy (per NeuronCore, data-parallel over batch, 8 images/core):
  - Zero-padded flat layout: each image is sign-binarized (bf16) into a
    [64, 114*114] SBUF tile with zero borders; every 3x3 tap then becomes a
    constant column offset, so the conv is 9 PSUM-accumulated matmuls
    (K=Cin=64, M=Cout=64) per 4-row output block.
  - Two images resident at once (partitions 0-63 / 64-127). Matmuls are
    issued with explicit tile_position so the 4 (image x row-half) streams
    occupy the 4 PE 64x64 quadrants CONCURRENTLY (measured ~10x vs serial).
  - Output rows are split top-half/bottom-half (rh) so each image's result
    stages as [128=(rh,c), 56*112] fp16 and leaves in ONE 1.6MB DMA with
    12.5KB contiguous per partition. fp16 is exact: outputs are integers
    bounded by 576 < 2048.
  - alpha is folded into the (ternary, exactly bf16-representable) weights.
  - x ships to the device as bf16 (host recode): bf16 shares the f32
    exponent range, so sign(x_bf16) == sign(x) element-exact, and input
    HBM/fabric traffic halves (the kernel was DMA-co-limited at f32).
"""

import numpy as np
import ml_dtypes

H = W = 112
WP = 114
P_COLS = WP * WP + 2  # 12998: +1 margin at each end
CIN = COUT = 64
N_CORES = 8
NI = 8  # images per core
ROWS_PER_CHUNK = 28  # input load/sign granularity
NJ = 14  # 4-row blocks per output half (56 rows per half)
USE_V2 = False  # fp8 DoubleRow variant (DR blocked on col groups 64+)


def build_nc(ni=NI, rep=1, skip_mm=False, skip_out=False, skip_in=False,
             bf16_x=True):
    import concourse.bacc as bacc
    import concourse.mybir as mybir
    from concourse.tile import TileContext

    f32 = mybir.dt.float32
    bf16 = mybir.dt.bfloat16
    fp16 = mybir.dt.float16

    x_dt = bf16 if bf16_x else f32
    nc = bacc.Bacc("TRN2", target_bir_lowering=False, debug=False)
    x_d = nc.dram_tensor("x", [ni, CIN, H, W], x_dt, kind="ExternalInput")
    w_d = nc.dram_tensor("w", [128, 9 * COUT], bf16, kind="ExternalInput")
    o_d = nc.dram_tensor("out", [ni, COUT, H, W], fp16, kind="ExternalOutput")

    x_flat = x_d.ap().rearrange("n c h w -> (n c) (h w)")
    npairs = ni // 2
    n_chunks = H // ROWS_PER_CHUNK  # 4

    with TileContext(nc) as tc:
        with (
            tc.tile_pool(name="wp", bufs=1) as wp,
            tc.tile_pool(name="inp", bufs=3) as inp,
            tc.tile_pool(name="pp", bufs=1) as pp,
            tc.tile_pool(name="op", bufs=2) as op,
            tc.tile_pool(name="psp", bufs=8, space="PSUM") as psp,
        ):
            w_sb = wp.tile([128, 9 * COUT], bf16, name="w_sb")
            nc.sync.dma_start(out=w_sb[:, :], in_=w_d.ap())

            p_tiles = []
            for i in range(2):
                pt = pp.tile([128, P_COLS], bf16, tag=f"p{i}", name=f"p{i}")
                nc.vector.memset(pt[:, :], 0.0)
                p_tiles.append(pt)

            for r in range(rep):
                for pair in range(npairs):
                    p = p_tiles[pair % 2]
                    # ---- load x for both images, binarize into p ----
                    for ci in range(n_chunks if not skip_in else 0):
                        y0 = ci * ROWS_PER_CHUNK
                        st = inp.tile([128, ROWS_PER_CHUNK * W], x_dt,
                                      tag="xin", name="xin")
                        nc.sync.dma_start(
                            out=st[:, :],
                            in_=x_flat[pair * 128:(pair + 1) * 128,
                                       y0 * W:(y0 + ROWS_PER_CHUNK) * W],
                        )
                        dst = p[:, 116 + y0 * WP:
                                116 + y0 * WP + ROWS_PER_CHUNK * WP]
                        dst = dst.rearrange("q (r w) -> q r w", w=WP)[:, :, :W]
                        src = st[:, :].rearrange("q (r w) -> q r w", w=W)
                        nc.scalar.activation(
                            out=dst, in_=src,
                            func=mybir.ActivationFunctionType.Sign)

                    # ---- output staging: [128=(rh,c), 56*112] fp16/img ----
                    st_out = []
                    for ii in range(2):
                        so = op.tile([128, NJ * 4 * W], fp16,
                                     tag=f"so{ii}", name=f"so{ii}")
                        st_out.append(so)

                    # ---- conv: 14 j-blocks x (2 img x 2 rh) quadrants ----
                    for j in range(NJ):
                        q_tiles = []
                        if not skip_mm:
                            for img in range(2):
                                qt = psp.tile([128, 456], f32, tag="ps",
                                              name=f"ps{img}",
                                              padded_shape=[128, 512])
                                q_tiles.append(qt)
                        for t in range(9 if not skip_mm else 0):
                            ky, kx = divmod(t, 3)
                            first, last = (t == 0), (t == 8)
                            for img in range(2):
                                lhs = w_sb[64 * img:64 * (img + 1),
                                           t * 64:(t + 1) * 64]
                                for rh in range(2):
                                    y0 = 4 * j + 56 * rh
                                    s = (y0 + ky) * WP + kx
                                    nc.tensor.matmul(
                                        q_tiles[img][64 * rh:64 * (rh + 1),
                                                     0:456],
                                        lhs,
                                        p[64 * img:64 * (img + 1), s:s + 456],
                                        start=first, stop=last,
                                        skip_group_check=True,
                                        tile_position=(64 * img, 64 * rh))
                        # evacuate both halves to fp16 staging (DVE)
                        if not skip_out:
                            for img in range(2):
                                if skip_mm:
                                    src = p[:, j * 456:(j + 1) * 456]
                                    src = src.rearrange("q (r w) -> q r w",
                                                        w=WP)[:, :, 1:1 + W]
                                else:
                                    src = q_tiles[img][:, 0:456]
                                    src = src.rearrange("q (r w) -> q r w",
                                                        w=WP)
                                    src = src[:, :, 1:1 + W]
                                dst = st_out[img][:, j * 4 * W:(j + 1) * 4 * W]
                                dst = dst.rearrange("q (r w) -> q r w", w=W)
                                nc.vector.tensor_copy(out=dst, in_=src)

                    # ---- DMA out: one per (image, row-half) ----
                    if not skip_out:
                        for img in range(2):
                            n = pair * 2 + img
                            for rh in range(2):
                                dst = o_d.ap()[n][:, 56 * rh:56 * (rh + 1), :]
                                dst = dst.rearrange("c r w -> c (r w)")
                                nc.sync.dma_start(
                                    out=dst,
                                    in_=st_out[img][64 * rh:64 * (rh + 1), :])
    nc.compile()
    return nc


def build_nc_v2(ni=NI, rep=1, skip_mm=False, skip_out=False, skip_in=False,
                act_every=4, dr=True):
    """fp8 DoubleRow variant: 4 paired-tap DR matmuls + 1 single per block.

    - input DMA casts f32->bf16 (SWDGE), ACT sign bf16->fp8e4 into p
    - weights fp8e4 [128, 9*64] (no alpha fold); alpha applied in evacuation
    - evacuation split DVE (tensor_scalar mul) / ACT (activation Copy+scale),
      ACT takes every `act_every`-th copy
    """
    import concourse.bacc as bacc
    import concourse.mybir as mybir
    from concourse.tile import TileContext
    from concourse.ap import AP

    f32 = mybir.dt.float32
    bf16 = mybir.dt.bfloat16
    fp16 = mybir.dt.float16
    fp8 = mybir.dt.float8e4

    nc = bacc.Bacc("TRN2", target_bir_lowering=False, debug=False)
    x_d = nc.dram_tensor("x", [ni, CIN, H, W], f32, kind="ExternalInput")
    w_d = nc.dram_tensor("w", [128, 9 * COUT], fp8, kind="ExternalInput")
    a_d = nc.dram_tensor("a", [128, 1], f32, kind="ExternalInput")
    o_d = nc.dram_tensor("out", [ni, COUT, H, W], fp16, kind="ExternalOutput")

    x_flat = x_d.ap().rearrange("n c h w -> (n c) (h w)")
    npairs = ni // 2
    n_chunks = H // ROWS_PER_CHUNK  # 4

    # tap order: t = 3*ky + kx ; offset within flat image = ky*WP + kx
    # DR pairs (consecutive taps): (0,1) d=1, (2,3) d=WP-2, (4,5) d=1,
    # (6,7) d=1 ; single tap 8.
    PAIRS = [(0, 1), (2, WP - 2), (4, 1), (6, 1)]
    TAP_OFF = [(t // 3) * WP + (t % 3) for t in range(9)]

    with TileContext(nc) as tc:
        with (
            tc.tile_pool(name="wp", bufs=1) as wp,
            tc.tile_pool(name="inp", bufs=3) as inp,
            tc.tile_pool(name="pp", bufs=1) as pp,
            tc.tile_pool(name="op", bufs=2) as op,
            tc.tile_pool(name="psp", bufs=8, space="PSUM") as psp,
        ):
            w_sb = wp.tile([128, 9 * COUT], fp8, name="w_sb")
            nc.sync.dma_start(out=w_sb[:, :], in_=w_d.ap())
            a_sb = wp.tile([128, 1], f32, name="a_sb")
            nc.sync.dma_start(out=a_sb[:, :], in_=a_d.ap())

            p_tiles = []
            for i in range(2):
                pt = pp.tile([128, P_COLS], fp8, tag=f"p{i}", name=f"p{i}")
                nc.vector.memset(pt[:, :], 0.0)
                p_tiles.append(pt)

            copy_idx = 0
            for r in range(rep):
                for pair in range(npairs):
                    p = p_tiles[pair % 2]
                    # ---- load x (cast to bf16), binarize to fp8 into p ----
                    for ci in range(n_chunks if not skip_in else 0):
                        y0 = ci * ROWS_PER_CHUNK
                        st = inp.tile([128, ROWS_PER_CHUNK * W], bf16,
                                      tag="xin", name="xin")
                        nc.gpsimd.dma_start(
                            out=st[:, :],
                            in_=x_flat[pair * 128:(pair + 1) * 128,
                                       y0 * W:(y0 + ROWS_PER_CHUNK) * W],
                        )
                        dst = p[:, 116 + y0 * WP:
                                116 + y0 * WP + ROWS_PER_CHUNK * WP]
                        dst = dst.rearrange("q (r w) -> q r w", w=WP)[:, :, :W]
                        src = st[:, :].rearrange("q (r w) -> q r w", w=W)
                        nc.scalar.activation(
                            out=dst, in_=src,
                            func=mybir.ActivationFunctionType.Sign)

                    # ---- output staging: [128=(rh,c), 56*112] fp16/img ----
                    st_out = []
                    for ii in range(2):
                        so = op.tile([128, NJ * 4 * W], fp16,
                                     tag=f"so{ii}", name=f"so{ii}")
                        st_out.append(so)

                    # ---- conv: 14 j-blocks x (2 img x 2 rh) quadrants ----
                    for j in range(NJ):
                        q_tiles = []
                        if not skip_mm:
                            for img in range(2):
                                qt = psp.tile([128, 456], f32, tag="ps",
                                              name=f"ps{img}",
                                              padded_shape=[128, 512])
                                q_tiles.append(qt)
                            for img in range(2):
                                for rh in range(2):
                                    y0 = 4 * j + 56 * rh
                                    s0 = y0 * WP
                                    pbase = img * 64 * P_COLS
                                    wbase = img * 64 * (9 * COUT)
                                    out_ap = q_tiles[img][
                                        64 * rh:64 * (rh + 1), 0:456]
                                    if dr:
                                        for pi, (t, d) in enumerate(PAIRS):
                                            rhs = AP(
                                                p.tensor,
                                                p[0:1, 0:1].offset + pbase
                                                + s0 + TAP_OFF[t],
                                                [[P_COLS, 64], [d, 2],
                                                 [1, 456]])
                                            lhsT = AP(
                                                w_sb.tensor,
                                                w_sb[0:1, 0:1].offset + wbase
                                                + t * COUT,
                                                [[9 * COUT, 64], [COUT, 2],
                                                 [1, COUT]])
                                            nc.tensor.matmul(
                                                out_ap, lhsT, rhs,
                                                start=(pi == 0), stop=False,
                                                perf_mode=(mybir.MatmulPerfMode
                                                           .DoubleRow),
                                                skip_group_check=True,
                                                tile_position=(64 * img,
                                                               64 * rh))
                                        lhsT8 = w_sb[64 * img:64 * (img + 1),
                                                     8 * COUT:9 * COUT]
                                        s8 = s0 + TAP_OFF[8]
                                        rhs8 = AP(
                                            p.tensor,
                                            p[0:1, 0:1].offset + pbase + s8,
                                            [[P_COLS, 64], [1, 456]])
                                        nc.tensor.matmul(
                                            out_ap, lhsT8, rhs8,
                                            start=False, stop=True,
                                            skip_group_check=True,
                                            tile_position=(64 * img, 64 * rh))
                                    else:
                                        for t in range(9):
                                            lhsTt = w_sb[
                                                64 * img:64 * (img + 1),
                                                t * COUT:(t + 1) * COUT]
                                            st_ = s0 + TAP_OFF[t]
                                            rhst = AP(
                                                p.tensor,
                                                p[0:1, 0:1].offset + pbase
                                                + st_,
                                                [[P_COLS, 64], [1, 456]])
                                            nc.tensor.matmul(
                                                out_ap, lhsTt, rhst,
                                                start=(t == 0), stop=(t == 8),
                                                skip_group_check=True,
                                                tile_position=(64 * img,
                                                               64 * rh))
                        # evacuate: alpha * psum -> fp16 staging (DVE/ACT mix)
                        if not skip_out:
                            for img in range(2):
                                if skip_mm:
                                    src = p[:, j * 456:(j + 1) * 456]
                                    src = src.rearrange("q (r w) -> q r w",
                                                        w=WP)[:, :, 1:1 + W]
                                else:
                                    src = q_tiles[img][:, 0:456]
                                    src = src.rearrange("q (r w) -> q r w",
                                                        w=WP)
                                    src = src[:, :, 1:1 + W]
                                dst = st_out[img][:, j * 4 * W:(j + 1) * 4 * W]
                                dst = dst.rearrange("q (r w) -> q r w", w=W)
                                if copy_idx % act_every == act_every - 1:
                                    nc.scalar.activation(
                                        out=dst, in_=src,
                                        func=(mybir.ActivationFunctionType
                                              .Copy),
                                        scale=a_sb[:, 0:1])
                                else:
                                    nc.vector.tensor_scalar_mul(
                                        dst, src, a_sb[:, 0:1])
                                copy_idx += 1

                    # ---- DMA out: one per (image, row-half) ----
                    if not skip_out:
                        for img in range(2):
                            n = pair * 2 + img
                            for rh in range(2):
                                dst = o_d.ap()[n][:, 56 * rh:56 * (rh + 1), :]
                                dst = dst.rearrange("c r w -> c (r w)")
                                nc.sync.dma_start(
                                    out=dst,
                                    in_=st_out[img][64 * rh:64 * (rh + 1), :])
    nc.compile()
    return nc


def pack_weights_v2(weight):
    """Ternarize (round(tanh(w))), pack as [128, 9*64] fp8e4 lhsT (no alpha)."""
    wt = _ternarize(np.asarray(weight, dtype=np.float32))
    arr = wt.transpose(1, 2, 3, 0).reshape(CIN, 9 * COUT)
    pack = np.empty((128, 9 * COUT), dtype=ml_dtypes.float8_e4m3)
    pack[0:64] = arr.astype(ml_dtypes.float8_e4m3)
    pack[64:128] = pack[0:64]
    return pack


def pack_alpha_v2(alpha):
    a = np.asarray(alpha, dtype=np.float32).reshape(-1)
    out = np.empty((128, 1), np.float32)
    out[0:64, 0] = a
    out[64:128, 0] = a
    return out


def pack_weights(weight, alpha):
    """Ternarize (round(tanh(w))), fold alpha, pack as [128, 9*64] bf16 lhsT."""
    wt = _ternarize(np.asarray(weight, dtype=np.float32))
    wt = wt * np.asarray(alpha, dtype=np.float32).reshape(-1, 1, 1, 1)
    # lhsT[k=cin, t*64+cout]
    arr = wt.transpose(1, 2, 3, 0).reshape(CIN, 9 * COUT)
    pack = np.empty((128, 9 * COUT), dtype=ml_dtypes.bfloat16)
    pack[0:64] = arr.astype(ml_dtypes.bfloat16)
    pack[64:128] = pack[0:64]
    return pack


def _ternarize(w):
    try:
        import jax
        cpu = jax.devices("cpu")[0]
        with jax.default_device(cpu):
            import jax.numpy as jnp
            return np.asarray(jnp.round(jnp.tanh(jnp.asarray(w))))
    except Exception:
        return np.round(np.tanh(w.astype(np.float32))).astype(np.float32)


_NC_CACHE = {}


def _get_nc(rep=1, **fl):
    key = ("nc", rep, tuple(sorted(fl.items())))
    if key not in _NC_CACHE:
        fl = dict(fl)
        builder = build_nc_v2 if fl.pop("v2", USE_V2) else build_nc
        _NC_CACHE[key] = builder(NI, rep=rep, **fl)
    return _NC_CACHE[key]


def _make_runner(rep=1, donate=True, **fl):
    """Build (once) a jitted shard_map callable running the NEFF on 8 cores."""
    key = ("runner", rep, donate, tuple(sorted(fl.items())))
    if key in _NC_CACHE:
        return _NC_CACHE[key]
    import jax
    import concourse.mybir as mybir
    from concourse import bass2jax
    from jax.sharding import Mesh, PartitionSpec
    from jax.experimental.shard_map import shard_map

    nc = _get_nc(rep, **fl)
    bass2jax.install_neuronx_cc_hook()

    partition_name = (nc.partition_id_tensor.name
                      if nc.partition_id_tensor else None)
    in_names, out_names, out_avals, zero_shapes = [], [], [], []
    for alloc in nc.m.functions[0].allocations:
        if not isinstance(alloc, mybir.MemoryLocationSet):
            continue
        name = alloc.memorylocations[0].name
        if alloc.kind == "ExternalInput":
            if name != partition_name:
                in_names.append(name)
        elif alloc.kind == "ExternalOutput":
            out_names.append(name)
            shape = tuple(alloc.tensor_shape)
            dtype = mybir.dt.np(alloc.dtype)
            out_avals.append(jax.core.ShapedArray(shape, dtype))
            zero_shapes.append((shape, dtype))
    n_params = len(in_names)
    all_in_names = in_names + out_names
    if partition_name is not None:
        all_in_names = all_in_names + [partition_name]

    def _body(*args):
        operands = list(args)
        if partition_name is not None:
            operands.append(bass2jax.partition_id_tensor())
        outs = bass2jax._bass_exec_p.bind(
            *operands,
            out_avals=tuple(out_avals),
            in_names=tuple(all_in_names),
            out_names=tuple(out_names),
            lowering_input_output_aliases=(),
            sim_require_finite=True,
            sim_require_nnan=True,
            nc=nc,
        )
        return tuple(outs)

    devices = jax.devices()[:N_CORES]
    mesh = Mesh(np.asarray(devices), ("core",))
    n_outs = len(out_names)
    donate_idx = tuple(range(n_params, n_params + n_outs)) if donate else ()
    in_specs = (PartitionSpec("core"),) * (n_params + n_outs)
    out_specs = (PartitionSpec("core"),) * n_outs
    sharded = jax.jit(
        shard_map(_body, mesh=mesh, in_specs=in_specs, out_specs=out_specs,
                  check_rep=False),
        donate_argnums=donate_idx, keep_unused=True)
    runner = {
        "fn": sharded, "mesh": mesh, "in_names": in_names,
        "out_names": out_names, "zero_shapes": zero_shapes,
        "n_params": n_params,
    }
    _NC_CACHE[key] = runner
    return runner


def make_concat_inputs(x, w_pack, a_pack=None):
    """Per-core inputs concatenated on axis 0 (shard_map layout)."""
    xs = np.ascontiguousarray(x.reshape(N_CORES * NI, CIN, H, W))
    ws = np.concatenate([w_pack] * N_CORES, axis=0)
    d = {"x": xs, "w": ws}
    if a_pack is not None:
        d["a"] = np.concatenate([a_pack] * N_CORES, axis=0)
    return d


def make_zeros(rep=1):
    r = _make_runner(rep)
    return [np.zeros((N_CORES * s[0], *s[1:]), d) for s, d in r["zero_shapes"]]


def run_concat(concat_by_name, zeros=None, rep=1):
    """Run on 8 cores. Inputs may be numpy or device-resident jax arrays."""
    r = _make_runner(rep)
    if zeros is None:
        zeros = make_zeros(rep)
    args = [concat_by_name[n] for n in r["in_names"]] + list(zeros)
    out_arrs = r["fn"](*args)
    return out_arrs


def make_concat_all(x, weight, alpha):
    """Concat inputs for the ACTIVE kernel variant (v1 or v2)."""
    x = np.asarray(x, dtype=np.float32)
    if USE_V2:
        return make_concat_inputs(x, pack_weights_v2(weight),
                                  pack_alpha_v2(alpha))
    # bf16 recode of x is sign-exact (bf16 keeps the f32 exponent range),
    # so the device computes the identical sign(x) conv.
    x = x.astype(ml_dtypes.bfloat16)
    return make_concat_inputs(x, pack_weights(weight, alpha))


def kernel(x, weight, alpha):
    concat = make_concat_all(x, weight, alpha)
    out_arrs = run_concat(concat)
    out = np.asarray(out_arrs[0]).reshape(64, COUT, H, W)
    return out.astype(np.float32)

